# revision 17
# baseline (speedup 1.0000x reference)
"""Locally-connected 2D layer on 8 Trainium2 NeuronCores.

Problem: x[128,3,64,64] f32, per-position weights W[60,60,32,75], bias b[60,60,32]
  out[b,o,y,x] = sum_k patches[b,y,x,k] * W[y,x,o,k] + b[y,x,o],  k=(c,dy,dx)

Strategy (spatial sharding over output rows, 8 rows/core, memory-regime):
  - Contraction (c,dy,dx)=75 (+1 bias) on SBUF partitions; dy handled by a
    mod-5 ring of patch planes XP[(r%5, c, dx), x*128+b]; per-row dy rotation
    folded into the host-side W layout (np.roll).
  - All tensors bf16 on the wire (PSUM accumulates f32; host widens output).
  - DMA packet size = per-partition contiguous bytes, and each DMA queue
    dispatches only ~1 packet per ~30-60ns, so everything is shaped for
    7680B packets: ring fills advance in column halves, outputs are stored
    in row pairs, W loads in 2-row pieces.
  - Bulk traffic is split across both hardware DGE queues (sync + scalar);
    W pieces use the gpsimd software queue plus spare early HW-queue time.
    The initial fill is column-quartered so row-0 matmuls start early.
  - Per output row: 5 column chunks x 12 matmuls (lhsT=W[76,32], rhs=
    XP[76,128] -> psum[32j:32j+32, 128]), DVE copy psum->bf16 SBUF pair
    tile, one 983KB store per row pair split across both HW queues.
"""

import numpy as np

B, C, H, WIDTH = 128, 3, 64, 64
KH = KW = 5
RY = RX = 60
O = 32
K = 75
NCORES = 8
RPC = 8             # output rows computed per core (8*8=64, last 4 dropped)
INR = RPC + KH - 1  # 12 input rows per core
PADH = NCORES * RPC + KH - 1  # 68
NG = 15             # groups of 4 x-positions per row
NCH = 5             # matmul column chunks per row (3 groups each)
GPC = 3             # groups per chunk
FXB = RX * B        # 7680 elements per patch plane
NPL = KH * C        # 15 planes per input row
WROW = RX * O       # 1920 weight elems per (row, partition)
NWP = 4             # W pieces (2 rows each)

_cache = {}


def _build():
    import concourse.bass as bass
    import concourse.bacc as bacc
    import concourse.tile as tile
    import concourse.mybir as mybir

    f32 = mybir.dt.float32
    bf16 = mybir.dt.bfloat16
    nc = bacc.Bacc("TRN2", target_bir_lowering=False, debug=False,
                   num_devices=NCORES)
    xpr_d = nc.dram_tensor("xpr", [INR, NPL, FXB], bf16, kind="ExternalInput")
    wh_d = nc.dram_tensor("wh", [NWP, K + 1, 2 * WROW], bf16,
                          kind="ExternalInput")
    ones_d = nc.dram_tensor("ones", [1, FXB], bf16, kind="ExternalInput")
    oc_d = nc.dram_tensor("oc", [RPC // 2, 128, 2 * NG * B], bf16,
                          kind="ExternalOutput")

    xr2 = xpr_d.rearrange("r p f -> (r p) f")  # [180, 7680]
    HFB = FXB // 2  # column half (3840 elems)
    QFB = FXB // 4  # column quarter (1920 elems)

    with tile.TileContext(nc) as tc:
        with (
            tc.tile_pool(name="const", bufs=1) as cpool,
            tc.tile_pool(name="w", bufs=4) as wpool,
            tc.tile_pool(name="os", bufs=3) as opool,
            tc.tile_pool(name="ps", bufs=4, space=bass.MemorySpace.PSUM) as ppool,
        ):
            xp = cpool.tile([K + 1, FXB], bf16)  # [76, 7680]; row 75 = ones

            nc.gpsimd.dma_start(xp[K:K + 1, :], ones_d[:])
            wts = {}

            def initq(qi, eng):
                eng.dma_start(xp[0:KH * NPL, qi * QFB:(qi + 1) * QFB],
                              xr2[0:KH * NPL, qi * QFB:(qi + 1) * QFB])

            def load_w(pc, eng):
                wts[pc] = wpool.tile([K + 1, 2 * WROW], bf16, name="wp",
                                     tag="wp")
                eng.dma_start(wts[pc][:], wh_d[pc])

            # prologue, all on the two HW DGE queues; FIFO order per queue
            # chosen so row-0 chunk-0's gates (init q0 + W row 0) land first
            w01 = wpool.tile([K + 1, 2 * WROW], bf16, name="wp", tag="wp")
            initq(0, nc.sync)
            initq(1, nc.scalar)
            nc.sync.dma_start(w01[:, 0:WROW], wh_d[0][:, 0:WROW])
            nc.scalar.dma_start(w01[:, WROW:2 * WROW], wh_d[0][:, WROW:2 * WROW])
            initq(2, nc.sync)
            initq(3, nc.scalar)
            wts[0] = w01
            load_w(1, nc.scalar)
            load_w(2, nc.sync)
            load_w(3, nc.scalar)

            for k in range(RPC):
                wpc = wts[k // 2]
                wrow = wpc[:, (k % 2) * WROW:(k % 2 + 1) * WROW]  # [76, 1920]
                if k % 2 == 0:
                    ot = opool.tile([128, 2 * NG * B], bf16)  # [128, 3840]
                ob = (k % 2) * NG * B
                for ci in range(NCH):
                    # full 2KB PSUM bank; only the first GPC*B cols are used
                    pt = ppool.tile([128, 4 * B], f32)
                    for gs in range(GPC):
                        for j in range(4):
                            xpos = (ci * GPC + gs) * 4 + j
                            nc.tensor.matmul(
                                pt[32 * j:32 * (j + 1), gs * B:(gs + 1) * B],
                                wrow[:, xpos * O:(xpos + 1) * O],
                                xp[:, xpos * B:(xpos + 1) * B],
                                tile_position=(0, 32 * j),
                            )
                    nc.vector.tensor_copy(
                        ot[:, ob + ci * GPC * B:ob + (ci + 1) * GPC * B],
                        pt[:, :GPC * B])
                    if k + KH < INR and ci in (2, 4):
                        # ring advance for row k+1 in column halves, gated on
                        # this row's matmul chunks that last read the slot
                        slot = k % KH
                        hf = 0 if ci == 2 else 1
                        f0, f1 = hf * HFB, (hf + 1) * HFB
                        eng = nc.sync if (k + hf) % 2 == 0 else nc.scalar
                        eng.dma_start(
                            xp[slot * NPL:(slot + 1) * NPL, f0:f1],
                            xpr_d[k + KH, :, f0:f1])
                if k % 2 == 1:
                    nc.sync.dma_start(oc_d[k // 2, 0:64], ot[0:64, :])
                    nc.scalar.dma_start(oc_d[k // 2, 64:128], ot[64:128, :])

    nc.compile()
    return nc


def _get_nc():
    if "nc" not in _cache:
        _cache["nc"] = _build()
    return _cache["nc"]


def _prep_inputs(x, W, b):
    import ml_dtypes
    bf = ml_dtypes.bfloat16
    x = np.asarray(x, np.float32)
    W = np.asarray(W, np.float32)
    b = np.asarray(b, np.float32)
    xh = np.zeros((PADH, C, WIDTH, B), np.float32)
    xh[:H] = x.transpose(2, 1, 3, 0)  # [row, c, w, batch]
    # ring planes: xpr_full[r, (c,dx) -> c*KW+dx, x, b] = xh[r, c, x+dx, b]
    xpr_full = np.zeros((PADH, C, KW, RX, B), np.float32)
    for dx in range(KW):
        xpr_full[:, :, dx] = xh[:, :, dx:dx + RX]
    xpr_full = xpr_full.reshape(PADH, C * KW, FXB)
    Wfull = W.transpose(0, 3, 1, 2)  # [RY, K, RX, O]
    in_maps = []
    for i in range(NCORES):
        whc = np.zeros((RPC, K + 1, RX, O), np.float32)
        for k in range(RPC):
            y = RPC * i + k
            if y < RY:
                w5 = Wfull[y].reshape(C, KH, KW, RX, O)
                # device slot rm holds input row with (local row)%5 == rm;
                # slot rm supplies dy=(rm-k)%5 for output row k -> roll by k.
                # partition order: p = rm*15 + c*5 + dx
                whc[k, :K] = np.roll(w5, k, axis=1).transpose(1, 0, 2, 3, 4) \
                    .reshape(K, RX, O)
                whc[k, K] = b[y]
        # piece layout: [piece, partition, (row_in_piece, WROW)]
        wres = np.ascontiguousarray(
            whc.reshape(NWP, 2, K + 1, WROW).transpose(0, 2, 1, 3)
               .reshape(NWP, K + 1, 2 * WROW))
        in_maps.append({
            "xpr": np.ascontiguousarray(
                xpr_full[RPC * i:RPC * i + INR]).astype(bf),
            "wh": wres.astype(bf),
            "ones": np.ones((1, FXB), bf),
        })
    return in_maps


def kernel(x, W, b):
    from concourse.bass_utils import run_bass_kernel_spmd

    nc = _get_nc()
    in_maps = _prep_inputs(x, W, b)
    br = run_bass_kernel_spmd(nc, in_maps, list(range(NCORES)),
                              **_cache.get("run_kwargs", {}))
    _cache["last_run"] = br
    oc = np.stack([np.asarray(br.results[i]["oc"]).astype(np.float32)
                   for i in range(NCORES)])
    # oc: [core, pair, (j,o), (k2, g, b)]
    oc = oc.reshape(NCORES * RPC // 2, 4, O, 2, NG, B)
    # out[b, o, row=pair*2+k2, x=g*4+j]
    oc = oc.transpose(5, 2, 0, 3, 4, 1)  # [b, o, pair, k2, g, j]
    out = oc.reshape(B, O, NCORES * RPC // 2 * 2, NG * 4)
    return np.ascontiguousarray(out[:, :, :RY, :])


# revision 21
# speedup vs baseline: 1.0051x; 1.0051x over previous
"""Locally-connected 2D layer on 8 Trainium2 NeuronCores.

Problem: x[128,3,64,64] f32, per-position weights W[60,60,32,75], bias b[60,60,32]
  out[b,o,y,x] = sum_k patches[b,y,x,k] * W[y,x,o,k] + b[y,x,o],  k=(c,dy,dx)

Strategy (spatial sharding over output rows, 8 rows/core, memory-regime):
  - Contraction (c,dy,dx)=75 (+1 bias) on SBUF partitions; dy handled by a
    mod-5 ring of patch planes XP[(r%5, c, dx), x*128+b]; per-row dy rotation
    folded into the host-side W layout (np.roll).
  - All tensors bf16 on the wire (PSUM accumulates f32; host widens output).
  - DMA packet size = per-partition contiguous bytes, and each DMA queue
    dispatches only ~1 packet per ~30-60ns, so everything is shaped for
    7680B packets: ring fills advance in column halves, outputs are stored
    in row pairs, W loads in 2-row pieces.
  - Bulk traffic is split across both hardware DGE queues (sync + scalar);
    W pieces use the gpsimd software queue plus spare early HW-queue time.
    The initial fill is column-quartered so row-0 matmuls start early.
  - Per output row: 5 column chunks x 12 matmuls (lhsT=W[76,32], rhs=
    XP[76,128] -> psum[32j:32j+32, 128]), DVE copy psum->bf16 SBUF pair
    tile, one 983KB store per row pair split across both HW queues.
"""

import numpy as np

B, C, H, WIDTH = 128, 3, 64, 64
KH = KW = 5
RY = RX = 60
O = 32
K = 75
NCORES = 8
RPC = 8             # output rows computed per core (8*8=64, last 4 dropped)
INR = RPC + KH - 1  # 12 input rows per core
PADH = NCORES * RPC + KH - 1  # 68
NG = 15             # groups of 4 x-positions per row
NCH = 5             # matmul column chunks per row (3 groups each)
GPC = 3             # groups per chunk
CW = GPC * 4 * B    # xp column width per chunk (1536 elements)
FXB = RX * B        # 7680 elements per patch plane
NPL = KH * C        # 15 planes per input row
WROW = RX * O       # 1920 weight elems per (row, partition)
NWP = 4             # W pieces (2 rows each)

_cache = {}


def _build():
    import concourse.bass as bass
    import concourse.bacc as bacc
    import concourse.tile as tile
    import concourse.mybir as mybir

    f32 = mybir.dt.float32
    bf16 = mybir.dt.bfloat16
    nc = bacc.Bacc("TRN2", target_bir_lowering=False, debug=False,
                   num_devices=NCORES)
    xpr_d = nc.dram_tensor("xpr", [INR, NPL, FXB], bf16, kind="ExternalInput")
    wh_d = nc.dram_tensor("wh", [NWP, K + 1, 2 * WROW], bf16,
                          kind="ExternalInput")
    ones_d = nc.dram_tensor("ones", [1, FXB], bf16, kind="ExternalInput")
    oc_d = nc.dram_tensor("oc", [RPC // 2, 128, 2 * NG * B], bf16,
                          kind="ExternalOutput")

    xr2 = xpr_d.rearrange("r p f -> (r p) f")  # [180, 7680]
    HFB = FXB // 2  # column half (3840 elems)
    QFB = FXB // 4  # column quarter (1920 elems)

    with tile.TileContext(nc) as tc:
        with (
            tc.tile_pool(name="const", bufs=1) as cpool,
            tc.tile_pool(name="w", bufs=4) as wpool,
            tc.tile_pool(name="os", bufs=3) as opool,
            tc.tile_pool(name="ps", bufs=6, space=bass.MemorySpace.PSUM) as ppool,
        ):
            xp = cpool.tile([K + 1, FXB], bf16)  # [76, 7680]; row 75 = ones

            wts = {}

            def initq(qi, eng):
                eng.dma_start(xp[0:KH * NPL, qi * QFB:(qi + 1) * QFB],
                              xr2[0:KH * NPL, qi * QFB:(qi + 1) * QFB])

            def load_w(pc, eng):
                wts[pc] = wpool.tile([K + 1, 2 * WROW], bf16, name="wp",
                                     tag="wp")
                eng.dma_start(wts[pc][:], wh_d[pc])

            # prologue, all on the two HW DGE queues; FIFO order per queue
            # chosen so row-0 chunk-0's gates (init q0 + W row 0) land first
            w01 = wpool.tile([K + 1, 2 * WROW], bf16, name="wp", tag="wp")
            nc.sync.dma_start(xp[K:K + 1, :], ones_d[:])
            nc.scalar.dma_start(w01[:, 0:WROW], wh_d[0][:, 0:WROW])
            initq(0, nc.sync)
            initq(1, nc.scalar)
            initq(2, nc.sync)
            initq(3, nc.scalar)
            nc.sync.dma_start(w01[:, WROW:2 * WROW], wh_d[0][:, WROW:2 * WROW])
            wts[0] = w01
            load_w(1, nc.scalar)
            load_w(2, nc.sync)
            load_w(3, nc.scalar)

            for k in range(RPC):
                wpc = wts[k // 2]
                wrow = wpc[:, (k % 2) * WROW:(k % 2 + 1) * WROW]  # [76, 1920]
                if k % 2 == 0:
                    ot = opool.tile([128, 2 * NG * B], bf16)  # [128, 3840]
                ob = (k % 2) * NG * B
                for ci in range(NCH):
                    # full 2KB PSUM bank; only the first GPC*B cols are used
                    pt = ppool.tile([128, 4 * B], f32)
                    for gs in range(GPC):
                        for j in range(4):
                            xpos = (ci * GPC + gs) * 4 + j
                            nc.tensor.matmul(
                                pt[32 * j:32 * (j + 1), gs * B:(gs + 1) * B],
                                wrow[:, xpos * O:(xpos + 1) * O],
                                xp[:, xpos * B:(xpos + 1) * B],
                                tile_position=(0, 32 * j),
                            )
                    nc.vector.tensor_copy(
                        ot[:, ob + ci * GPC * B:ob + (ci + 1) * GPC * B],
                        pt[:, :GPC * B])
                    if k + KH < INR:
                        # ring advance for row k+1, chunk by chunk, each
                        # gated only on this chunk's matmuls so the fill of
                        # chunk ci overlaps chunks ci+1.. of this row
                        slot = k % KH
                        f0, f1 = ci * CW, (ci + 1) * CW
                        eng = nc.sync if (k + ci) % 2 == 0 else nc.scalar
                        eng.dma_start(
                            xp[slot * NPL:(slot + 1) * NPL, f0:f1],
                            xpr_d[k + KH, :, f0:f1])
                if k % 2 == 1:
                    nc.sync.dma_start(oc_d[k // 2, 0:64], ot[0:64, :])
                    nc.scalar.dma_start(oc_d[k // 2, 64:128], ot[64:128, :])

    nc.compile()
    return nc


def _get_nc():
    if "nc" not in _cache:
        _cache["nc"] = _build()
    return _cache["nc"]


def _prep_inputs(x, W, b):
    import ml_dtypes
    bf = ml_dtypes.bfloat16
    x = np.asarray(x, np.float32)
    W = np.asarray(W, np.float32)
    b = np.asarray(b, np.float32)
    xh = np.zeros((PADH, C, WIDTH, B), np.float32)
    xh[:H] = x.transpose(2, 1, 3, 0)  # [row, c, w, batch]
    # ring planes: xpr_full[r, (c,dx) -> c*KW+dx, x, b] = xh[r, c, x+dx, b]
    xpr_full = np.zeros((PADH, C, KW, RX, B), np.float32)
    for dx in range(KW):
        xpr_full[:, :, dx] = xh[:, :, dx:dx + RX]
    xpr_full = xpr_full.reshape(PADH, C * KW, FXB)
    Wfull = W.transpose(0, 3, 1, 2)  # [RY, K, RX, O]
    in_maps = []
    for i in range(NCORES):
        whc = np.zeros((RPC, K + 1, RX, O), np.float32)
        for k in range(RPC):
            y = RPC * i + k
            if y < RY:
                w5 = Wfull[y].reshape(C, KH, KW, RX, O)
                # device slot rm holds input row with (local row)%5 == rm;
                # slot rm supplies dy=(rm-k)%5 for output row k -> roll by k.
                # partition order: p = rm*15 + c*5 + dx
                whc[k, :K] = np.roll(w5, k, axis=1).transpose(1, 0, 2, 3, 4) \
                    .reshape(K, RX, O)
                whc[k, K] = b[y]
        # piece layout: [piece, partition, (row_in_piece, WROW)]
        wres = np.ascontiguousarray(
            whc.reshape(NWP, 2, K + 1, WROW).transpose(0, 2, 1, 3)
               .reshape(NWP, K + 1, 2 * WROW))
        in_maps.append({
            "xpr": np.ascontiguousarray(
                xpr_full[RPC * i:RPC * i + INR]).astype(bf),
            "wh": wres.astype(bf),
            "ones": np.ones((1, FXB), bf),
        })
    return in_maps


def kernel(x, W, b):
    from concourse.bass_utils import run_bass_kernel_spmd

    nc = _get_nc()
    in_maps = _prep_inputs(x, W, b)
    br = run_bass_kernel_spmd(nc, in_maps, list(range(NCORES)),
                              **_cache.get("run_kwargs", {}))
    _cache["last_run"] = br
    oc = np.stack([np.asarray(br.results[i]["oc"]).astype(np.float32)
                   for i in range(NCORES)])
    # oc: [core, pair, (j,o), (k2, g, b)]
    oc = oc.reshape(NCORES * RPC // 2, 4, O, 2, NG, B)
    # out[b, o, row=pair*2+k2, x=g*4+j]
    oc = oc.transpose(5, 2, 0, 3, 4, 1)  # [b, o, pair, k2, g, j]
    out = oc.reshape(B, O, NCORES * RPC // 2 * 2, NG * 4)
    return np.ascontiguousarray(out[:, :, :RY, :])


# revision 25
# speedup vs baseline: 1.0461x; 1.0409x over previous
"""Locally-connected 2D layer on 8 Trainium2 NeuronCores.

Problem: x[128,3,64,64] f32, per-position weights W[60,60,32,75], bias b[60,60,32]
  out[b,o,y,x] = sum_k patches[b,y,x,k] * W[y,x,o,k] + b[y,x,o],  k=(c,dy,dx)

Strategy (spatial sharding over output rows, 8 rows/core, memory-regime):
  - Contraction (c,dy,dx)=75 (+1 bias) on SBUF partitions; dy handled by a
    mod-5 ring of patch planes XP[(r%5, c, dx), x*128+b]; per-row dy rotation
    folded into the host-side W layout (np.roll).
  - All tensors bf16 on the wire (PSUM accumulates f32; host widens output).
  - DMA packet size = per-partition contiguous bytes, and each DMA queue
    dispatches only ~1 packet per ~30-60ns, so everything is shaped for
    7680B packets: ring fills advance in column halves, outputs are stored
    in row pairs, W loads in 2-row pieces.
  - Bulk traffic is split across both hardware DGE queues (sync + scalar);
    W pieces use the gpsimd software queue plus spare early HW-queue time.
    The initial fill is column-quartered so row-0 matmuls start early.
  - Per output row: 5 column chunks x 12 matmuls (lhsT=W[76,32], rhs=
    XP[76,128] -> psum[32j:32j+32, 128]), DVE copy psum->bf16 SBUF pair
    tile, one 983KB store per row pair split across both HW queues.
"""

import numpy as np

B, C, H, WIDTH = 128, 3, 64, 64
KH = KW = 5
RY = RX = 60
O = 32
K = 75
NCORES = 8
RPC = 8             # output rows computed per core (8*8=64, last 4 dropped)
INR = RPC + KH - 1  # 12 input rows per core
PADH = NCORES * RPC + KH - 1  # 68
NG = 15             # groups of 4 x-positions per row
NCH = 5             # matmul column chunks per row (3 groups each)
GPC = 3             # groups per chunk
CW = GPC * 4 * B    # xp column width per chunk (1536 elements)
FXB = RX * B        # 7680 elements per patch plane
NPL = KH * C        # 15 planes per input row
WROW = RX * O       # 1920 weight elems per (row, partition)
NWP = 4             # W pieces (2 rows each)

_cache = {}


def _build():
    import concourse.bass as bass
    import concourse.bacc as bacc
    import concourse.tile as tile
    import concourse.mybir as mybir

    f32 = mybir.dt.float32
    bf16 = mybir.dt.bfloat16
    nc = bacc.Bacc("TRN2", target_bir_lowering=False, debug=False,
                   num_devices=NCORES)
    xpr_d = nc.dram_tensor("xpr", [INR, NPL, FXB], bf16, kind="ExternalInput")
    wh_d = nc.dram_tensor("wh", [NWP, K + 1, 2 * WROW], bf16,
                          kind="ExternalInput")
    ones_d = nc.dram_tensor("ones", [1, FXB], bf16, kind="ExternalInput")
    oc_d = nc.dram_tensor("oc", [RPC // 2, 128, 2 * NG * B], bf16,
                          kind="ExternalOutput")

    xr2 = xpr_d.rearrange("r p f -> (r p) f")  # [180, 7680]
    HFB = FXB // 2  # column half (3840 elems)
    QFB = FXB // 4  # column quarter (1920 elems)

    with tile.TileContext(nc) as tc:
        with (
            tc.tile_pool(name="const", bufs=1) as cpool,
            tc.tile_pool(name="w", bufs=4) as wpool,
            tc.tile_pool(name="os", bufs=3) as opool,
            tc.tile_pool(name="ps", bufs=6, space=bass.MemorySpace.PSUM) as ppool,
        ):
            xp = cpool.tile([K + 1, FXB], bf16)  # [76, 7680]; row 75 = ones

            wts = {}

            def initq(qi, eng):
                # column halves -> 7680B per-partition packets
                eng.dma_start(xp[0:KH * NPL, qi * HFB:(qi + 1) * HFB],
                              xr2[0:KH * NPL, qi * HFB:(qi + 1) * HFB])

            def load_w(pc, eng):
                wts[pc] = wpool.tile([K + 1, 2 * WROW], bf16, name="wp",
                                     tag="wp")
                eng.dma_start(wts[pc][:], wh_d[pc])

            # prologue: init fill + W rows 0-1 on the two HW DGE queues
            # (FIFO order puts row-0 chunk-0's gates first); W rows 2-7
            # prefetch on the gpsimd software queue so they never sit in a
            # HW queue ahead of the ring fills
            w01 = wpool.tile([K + 1, 2 * WROW], bf16, name="wp", tag="wp")
            nc.sync.dma_start(xp[K:K + 1, :], ones_d[:])
            nc.scalar.dma_start(w01[:, 0:WROW], wh_d[0][:, 0:WROW])
            initq(0, nc.sync)
            initq(1, nc.scalar)
            nc.sync.dma_start(w01[:, WROW:2 * WROW], wh_d[0][:, WROW:2 * WROW])
            wts[0] = w01
            load_w(1, nc.gpsimd)
            load_w(2, nc.gpsimd)
            load_w(3, nc.gpsimd)

            for k in range(RPC):
                wpc = wts[k // 2]
                wrow = wpc[:, (k % 2) * WROW:(k % 2 + 1) * WROW]  # [76, 1920]
                if k % 2 == 0:
                    ot = opool.tile([128, 2 * NG * B], bf16)  # [128, 3840]
                ob = (k % 2) * NG * B
                for ci in range(NCH):
                    # full 2KB PSUM bank; only the first GPC*B cols are used
                    pt = ppool.tile([128, 4 * B], f32)
                    for gs in range(GPC):
                        for j in range(4):
                            xpos = (ci * GPC + gs) * 4 + j
                            nc.tensor.matmul(
                                pt[32 * j:32 * (j + 1), gs * B:(gs + 1) * B],
                                wrow[:, xpos * O:(xpos + 1) * O],
                                xp[:, xpos * B:(xpos + 1) * B],
                                tile_position=(0, 32 * j),
                            )
                    nc.vector.tensor_copy(
                        ot[:, ob + ci * GPC * B:ob + (ci + 1) * GPC * B],
                        pt[:, :GPC * B])
                    if k + KH < INR and ci in (2, 4):
                        # ring advance for row k+1 in column halves with
                        # 7680B packets: h0 fires after chunk 2 (overlapping
                        # chunks 3-4), h1 after chunk 4
                        slot = k % KH
                        hf = 0 if ci == 2 else 1
                        f0, f1 = hf * HFB, (hf + 1) * HFB
                        eng = nc.sync if (k + hf) % 2 == 0 else nc.scalar
                        eng.dma_start(
                            xp[slot * NPL:(slot + 1) * NPL, f0:f1],
                            xpr_d[k + KH, :, f0:f1])
                if k % 2 == 1:
                    nc.sync.dma_start(oc_d[k // 2, 0:64], ot[0:64, :])
                    nc.scalar.dma_start(oc_d[k // 2, 64:128], ot[64:128, :])

    nc.compile()
    return nc


def _get_nc():
    if "nc" not in _cache:
        _cache["nc"] = _build()
    return _cache["nc"]


def _prep_inputs(x, W, b):
    import ml_dtypes
    bf = ml_dtypes.bfloat16
    x = np.asarray(x, np.float32)
    W = np.asarray(W, np.float32)
    b = np.asarray(b, np.float32)
    xh = np.zeros((PADH, C, WIDTH, B), np.float32)
    xh[:H] = x.transpose(2, 1, 3, 0)  # [row, c, w, batch]
    # ring planes: xpr_full[r, (c,dx) -> c*KW+dx, x, b] = xh[r, c, x+dx, b]
    xpr_full = np.zeros((PADH, C, KW, RX, B), np.float32)
    for dx in range(KW):
        xpr_full[:, :, dx] = xh[:, :, dx:dx + RX]
    xpr_full = xpr_full.reshape(PADH, C * KW, FXB)
    Wfull = W.transpose(0, 3, 1, 2)  # [RY, K, RX, O]
    in_maps = []
    for i in range(NCORES):
        whc = np.zeros((RPC, K + 1, RX, O), np.float32)
        for k in range(RPC):
            y = RPC * i + k
            if y < RY:
                w5 = Wfull[y].reshape(C, KH, KW, RX, O)
                # device slot rm holds input row with (local row)%5 == rm;
                # slot rm supplies dy=(rm-k)%5 for output row k -> roll by k.
                # partition order: p = rm*15 + c*5 + dx
                whc[k, :K] = np.roll(w5, k, axis=1).transpose(1, 0, 2, 3, 4) \
                    .reshape(K, RX, O)
                whc[k, K] = b[y]
        # piece layout: [piece, partition, (row_in_piece, WROW)]
        wres = np.ascontiguousarray(
            whc.reshape(NWP, 2, K + 1, WROW).transpose(0, 2, 1, 3)
               .reshape(NWP, K + 1, 2 * WROW))
        in_maps.append({
            "xpr": np.ascontiguousarray(
                xpr_full[RPC * i:RPC * i + INR]).astype(bf),
            "wh": wres.astype(bf),
            "ones": np.ones((1, FXB), bf),
        })
    return in_maps


def kernel(x, W, b):
    from concourse.bass_utils import run_bass_kernel_spmd

    nc = _get_nc()
    in_maps = _prep_inputs(x, W, b)
    br = run_bass_kernel_spmd(nc, in_maps, list(range(NCORES)),
                              **_cache.get("run_kwargs", {}))
    _cache["last_run"] = br
    oc = np.stack([np.asarray(br.results[i]["oc"]).astype(np.float32)
                   for i in range(NCORES)])
    # oc: [core, pair, (j,o), (k2, g, b)]
    oc = oc.reshape(NCORES * RPC // 2, 4, O, 2, NG, B)
    # out[b, o, row=pair*2+k2, x=g*4+j]
    oc = oc.transpose(5, 2, 0, 3, 4, 1)  # [b, o, pair, k2, g, j]
    out = oc.reshape(B, O, NCORES * RPC // 2 * 2, NG * 4)
    return np.ascontiguousarray(out[:, :, :RY, :])
